# revision 29
# baseline (speedup 1.0000x reference)
"""AdaptiveCenterLoss on 8 TRN2 NeuronCores.

loss = mean_i ||features[i] - centers[labels[i]]||^2
     with B=131072, D=256, C=1000.

Strategy (data-parallel, memory-bound):
  - host-side, sort rows by label and pack them into one-label blocks of
    32 bulk rows (remainders go to one 16- or 8-row block); partial
    blocks are padded with rows equal to that class's center,
    contributing exactly 0 to the sum.
  - features and centers are cast to bf16 on the host: the kernel is
    HBM-bandwidth-bound and the 2e-2 tolerance leaves orders of
    magnitude of headroom (measured rel err ~2e-5), so halving the
    bytes halves the DMA wall.
  - each block's DRAM line is [center row | block rows]: the center
    ships inside the same per-partition descriptor as the features, so
    there is NO indirect gather, no labels tensor, and no GpSimd DGE
    software cost on the device (per-tile indirect gathers all landed
    on DMA queues 0-3 and made them the bottleneck).
  - blocks are sharded across 8 cores into tiles of up to 128 blocks
    (one 16.9KB line-descriptor per partition); every tile gets its own
    SBUF buffer so all DMA triggers issue upfront and the 16 queues
    stream back-to-back with no buffer-recycling waits.
  - small/ragged tiles are emitted FIRST so the DVE/ACT pipeline warms
    up during the slow first ~10us of HBM delivery, and the last tile's
    ACT/DVE split is shifted toward DVE so both engines drain together.
  - per sub-tile: DVE subtracts the in-line center (broadcast over
    slots, 0.538 ns/elem); the square+row-sum is SPLIT between ACT
    (Square+accum, 0.833 ns/elem + ~670ns fixed) and DVE
    (scalar_tensor_tensor mult+mult with accum_out, 1.08 ns/elem), so
    both engines finish a 2MB sub-tile in ~6.0us, at par with the
    ~330 GB/s aggregate DMA pace.
  - each core outputs per-block partial sums; host sums and divides by B
"""

import numpy as np
import ml_dtypes

import concourse.bacc as bacc
import concourse.bass as bass  # noqa: F401
import concourse.mybir as mybir
import concourse.tile as tile
from concourse.bass_utils import run_bass_kernel_spmd

B, D, C = 131072, 256, 1000
N_CORES = 8
P = 128

# block sizes, descending; remainder rows go to the smallest size that fits
BLOCK_SIZES = (32, 16, 8)

# elems per partition handed to ACT (rest to DVE STT), per slot count;
# balance of ACT 0.833x+670 vs DVE 0.538*n + 1.08*(n-x)
ACT_ELEMS = {32: 6656, 16: 3200, 8: 1472}

_nc_cache = {}


def _build(units):
    """Per-core graph; units = ((p, slots, nsub), ...): one DMA per unit,
    nsub sub-tiles of p partitions x slots rows (+1 center line each)."""
    key = tuple(units)
    if key in _nc_cache:
        return _nc_cache[key]
    n_acc = 2 * sum(u[2] for u in units)
    rows_core = sum(p * nsub * (s + 1) for p, s, nsub, _ in units)

    nc = bacc.Bacc()
    feats = nc.declare_dram_parameter(
        "features", [rows_core, D], mybir.dt.bfloat16, isOutput=False
    )
    out = nc.declare_dram_parameter("out", [P, n_acc], mybir.dt.float32, isOutput=True)

    fall = feats[:]

    n_units = len(units)
    with tile.TileContext(nc) as tc:
        with (
            # one buffer per unit: every DMA trigger issues upfront with no
            # buffer-recycling waits (total ~118KB/partition, fits SBUF)
            tc.tile_pool(name="f", bufs=min(n_units, 12)) as f_pool,
            tc.tile_pool(name="acc", bufs=1) as acc_pool,
        ):
            acc = acc_pool.tile([P, n_acc], mybir.dt.float32)
            # ragged tiles leave partitions p..127 of their acc columns
            # unwritten; zero them so the final out DMA reads defined data
            nc.vector.memset(acc[:], 0.0)
            col = 0
            for ui, (p, slots, nsub, rowbase) in enumerate(units):
                lw = (slots + 1) * D  # elems per sub-tile line
                f_t = f_pool.tile([P, nsub * lw], mybir.dt.bfloat16, tag="f")
                if nsub == 1:
                    nc.sync.dma_start(
                        out=f_t[0:p, :].rearrange("p (s d) -> p s d", s=slots + 1),
                        in_=fall[rowbase : rowbase + p * (slots + 1), :].rearrange(
                            "(p s) d -> p s d", p=p
                        ),
                    )
                else:
                    nc.sync.dma_start(
                        out=f_t[0:p, :].rearrange(
                            "p (t s d) -> p t s d", t=nsub, s=slots + 1
                        ),
                        in_=fall[
                            rowbase : rowbase + p * nsub * (slots + 1), :
                        ].rearrange("(p t s) d -> p t s d", p=p, t=nsub),
                    )
                for t in range(nsub):
                    base = t * lw
                    w = base + lw
                    c_b = (
                        f_t[0:p, base : base + D]
                        .rearrange("p (s d) -> p s d", s=1)
                        .to_broadcast([p, slots, D])
                    )
                    nc.vector.tensor_tensor(
                        out=f_t[0:p, base + D : w].rearrange(
                            "p (s d) -> p s d", s=slots
                        ),
                        in0=f_t[0:p, base + D : w].rearrange(
                            "p (s d) -> p s d", s=slots
                        ),
                        in1=c_b,
                        op=mybir.AluOpType.subtract,
                    )
                    if slots != 32:
                        # warmup tiles run while DVE/ACT are otherwise idle
                        # during the HBM ramp: squares go entirely to ACT,
                        # freeing DVE to start the next subtract sooner
                        a = slots * D
                    elif ui == len(units) - 1:
                        # drain tile: shift squares toward DVE so the ACT
                        # tail after the last subtract is shorter
                        a = 4352
                    else:
                        a = ACT_ELEMS[slots]
                    nc.scalar.activation(
                        out=f_t[0:p, base + D : base + D + a],
                        in_=f_t[0:p, base + D : base + D + a],
                        func=mybir.ActivationFunctionType.Square,
                        accum_out=acc[0:p, col : col + 1],
                    )
                    if a < slots * D:
                        # (tensor_tensor_reduce crashes on this HW path; STT
                        # accum_out = same square+row-sum in one DVE op)
                        nc.vector.scalar_tensor_tensor(
                            out=f_t[0:p, base + D + a : w],
                            in0=f_t[0:p, base + D + a : w],
                            scalar=1.0,
                            in1=f_t[0:p, base + D + a : w],
                            op0=mybir.AluOpType.mult,
                            op1=mybir.AluOpType.mult,
                            accum_out=acc[0:p, col + 1 : col + 2],
                        )
                    col += 2
            nc.sync.dma_start(out=out[:], in_=acc[:])
    nc.finalize()
    _nc_cache[key] = nc
    return nc


def _prepare(features, centers, labels):
    features = np.ascontiguousarray(np.asarray(features), dtype=np.float32)
    centers = np.ascontiguousarray(np.asarray(centers), dtype=np.float32)
    labels = np.asarray(labels).astype(np.int32)

    counts = np.bincount(labels, minlength=C)
    S0 = BLOCK_SIZES[0]
    bulk = counts // S0
    rem = counts % S0
    # per-class block counts per size: remainder to the smallest fitting size
    bcnt = {s: np.zeros(C, dtype=np.int64) for s in BLOCK_SIZES}
    bcnt[S0] += bulk
    prev = 0
    for s in sorted(BLOCK_SIZES):
        bcnt[s] += (rem > prev) & (rem <= s)
        prev = s

    # force the 32-region to a multiple of 128 blocks/core by splitting the
    # remainder into pairs of 16-blocks (+512B/split): DMA queues are keyed
    # on partition//8, so a RAGGED 32-slot tile piles all its descriptors
    # onto the first few queues and stalls the stream start; after the
    # split the only wide early tile is a full-width balanced (128,16)
    N32 = int(bcnt[S0].sum())
    n32_tiles = (N32 // (N_CORES * P)) * N_CORES * P
    n_split = N32 - n32_tiles
    if n32_tiles and n_split:
        idx = np.where(bcnt[S0] >= 1)[0][:n_split]
        if len(idx) == n_split:
            bcnt[S0][idx] -= 1
            bcnt[16][idx] += 2

    n_core_of = {
        s: (-(-int(bcnt[s].sum()) // N_CORES) if bcnt[s].sum() else 0)
        for s in BLOCK_SIZES
    }
    # emission order: small/ragged warmup tiles first, full 32-tiles last
    tf32, pr32 = divmod(n_core_of[S0], P)
    chunks = []  # (size, blocks-per-core)
    if n_core_of[16]:
        chunks.append((16, n_core_of[16]))
    if n_core_of[8]:
        chunks.append((8, n_core_of[8]))
    if pr32:
        chunks.append((32, pr32))
    if tf32:
        chunks.append((32, tf32 * P))

    rows_core = sum((s + 1) * n for s, n in chunks)

    # units carry explicit row offsets so the emission order (ragged
    # high-work-per-byte warmups first, full-width tiles after) can
    # differ from the layout order; ragged part first WITHIN each chunk
    unit_warm, unit_mid, unit_full32 = [], [], []
    off16 = off8 = off32A = off32B = 0
    core_off = 0
    for s, n in chunks:
        if s == 16:
            off16 = core_off
        elif s == 8:
            off8 = core_off
        elif n == pr32 and s == S0:
            off32A = core_off
        else:
            off32B = core_off
        r = n % P
        nblk = 0
        if r:
            u = (r, s, 1, core_off)
            (unit_warm if s != S0 or n == pr32 else unit_full32).append(u)
            nblk = r
        while nblk < n:
            u = (P, s, 1, core_off + nblk * (s + 1))
            (unit_full32 if s == S0 else unit_mid).append(u)
            nblk += P
        core_off += (s + 1) * n
    units = unit_warm + unit_mid + unit_full32

    # per-size-region: class-major block labels and row starts in the
    # emission layout (region split across chunks sequentially per core)
    region_labs = {}
    region_rstart = {}
    for s in BLOCK_SIZES:
        n_core = n_core_of[s]
        if n_core == 0:
            region_labs[s] = np.zeros(0, np.int32)
            region_rstart[s] = np.zeros(0, np.int64)
            continue
        labs = np.zeros(N_CORES * n_core, dtype=np.int32)
        N = int(bcnt[s].sum())
        labs[:N] = np.repeat(np.arange(C, dtype=np.int32), bcnt[s])
        j = np.arange(N_CORES * n_core, dtype=np.int64)
        k = j // n_core
        jl = j % n_core  # core-local block index within this size's region
        if s == S0:
            # first pr32 blocks live in the ragged chunk, rest in fulls
            off = np.where(
                jl < pr32,
                off32A + jl * (s + 1),
                off32B + (jl - pr32) * (s + 1),
            )
        else:
            off = (off16 if s == 16 else off8) + jl * (s + 1)
        region_rstart[s] = k * rows_core + off
        region_labs[s] = labs

    # init every line with its block's center -> pad rows contribute 0
    fpad = np.empty((N_CORES * rows_core, D), dtype=np.float32)
    for s in BLOCK_SIZES:
        if n_core_of[s] == 0:
            continue
        rows = (region_rstart[s][:, None] + np.arange(s + 1)).ravel()
        fpad[rows] = centers[region_labs[s]].repeat(s + 1, axis=0)

    # scatter real rows: class-major rank -> (region, block, slot)
    order = np.argsort(labels)
    labels_sorted = labels[order]
    class_row_start = np.concatenate(([0], np.cumsum(counts)[:-1]))
    rank = np.arange(B) - class_row_start[labels_sorted]
    dst = np.empty(B, dtype=np.int64)
    assigned = np.zeros(B, dtype=bool)
    for s in BLOCK_SIZES:
        if n_core_of[s] == 0:
            continue
        start_s = np.concatenate(([0], np.cumsum(bcnt[s])[:-1]))
        cap = s * bcnt[s][labels_sorted]
        m = (~assigned) & (rank < cap)
        blk = start_s[labels_sorted[m]] + rank[m] // s
        dst[m] = region_rstart[s][blk] + 1 + rank[m] % s
        assigned |= m
        rank = rank - cap
    assert assigned.all()
    fpad[dst] = features[order]

    f16 = fpad.astype(ml_dtypes.bfloat16)
    maps = [
        {"features": f16[k * rows_core : (k + 1) * rows_core]}
        for k in range(N_CORES)
    ]
    return maps, tuple(units)


def _valid_subtiles(units):
    for p, slots, nsub, _roff in units:
        for _ in range(nsub):
            yield p, slots


def run(features, centers, labels, trace=False):
    maps, units = _prepare(features, centers, labels)
    nc = _build(units)
    res = run_bass_kernel_spmd(
        nc, maps, core_ids=list(range(N_CORES)), trace=trace
    )
    total = 0.0
    for r in res.results:
        o = np.asarray(r["out"]).astype(np.float64)
        for t, (p, _slots) in enumerate(_valid_subtiles(units)):
            total += o[0:p, 2 * t].sum() + o[0:p, 2 * t + 1].sum()
    return np.float32(total / B), res


def kernel(features, centers, labels):
    last_err = None
    for _ in range(3):
        try:
            loss, _ = run(features, centers, labels)
            return loss
        except Exception as e:  # noqa: BLE001
            last_err = e
    raise last_err
